# revision 1
# baseline (speedup 1.0000x reference)
"""Trainium2 Bass kernel for fused LayerNorm + multi-head ALiBi attention.

Reference computation (B=2, S=2048, D=1024, H=16 heads, dh=64):
    xn = LayerNorm(x) * gamma + beta
    q,k,v = split_heads(xn @ Wq), ... ; att = softmax(q k^T / 8 + alibi); out = (att v) @ Wo

Sharding: 8 cores = 2 batches x 4 head-groups (4 heads each).  Each core
computes a partial output (its heads' contribution through its Wo row-slice);
host sums the 4 partials per batch (the tensor-parallel all-reduce).

Per-core layout strategy ("transposed" attention):
  - xn is transposed on-chip to xnT [D, S] via PE-transpose; gamma/beta are
    applied during the PSUM->SBUF evacuation (per-partition scalars there).
  - Q,K projections produce Q^T/K^T [head_dim, S]; scores are computed
    directly as scores^T[j, i] tiles (j on partitions), which is exactly the
    layout the PV matmul wants as its moving operand -- no p-transposes.
  - ALiBi (-c_h|i-j|) is folded into the score matmul via 2 extra contraction
    rows: k' = [k, j, 1], q'_lower = [q, 8c, -8c*i], q'_upper = -aug.  Tiles
    crossing the diagonal are computed with both variants and combined with an
    elementwise min (since -|x| = min(x, -x)).
  - Softmax skips max-subtraction (alibi <= 0 and qk/8 is O(1), so exp never
    overflows); the row-sums come for free from an appended ones-column in V
    (PV psum row 64).  Normalization divides PV psum rows by row 64; any
    per-column factor on p~ cancels in that ratio.
  - Matmul operands are bitcast to float32r (FP22): full PE rate at N>=256.
    p~ and V are stored bf16 (errors average out in the PV contraction).
"""

import ml_dtypes
import numpy as np

import concourse.bass as bass
import concourse.tile as tile
from concourse import bacc, mybir
from concourse.bass_utils import run_bass_kernel_spmd
from concourse.masks import make_identity

F32 = mybir.dt.float32
F32R = mybir.dt.float32r
BF16 = mybir.dt.bfloat16
F16 = mybir.dt.float16
AF = mybir.ActivationFunctionType
OP = mybir.AluOpType

S = 2048
D = 1024
HD = 64          # head dim
NH = 4           # heads per core
INNER = NH * HD  # 256
P = 128
NTS = S // P     # 16 s-tiles
NDT = D // P     # 8 d-tiles
SI = 512         # i-tile width
NI = S // SI     # 4 i-tiles
NJT = S // P     # 16 j-tiles
KAUG = HD + 2    # 66 = augmented contraction for scores

_CACHE = {}

SKIP_THRESH = 30.0


def _c_of(hgl):
    return 2.0 ** (-8.0 / (16 - hgl))


def heads_of_group(g):
    """Round-robin head assignment: core group g takes heads g, g+4, g+8, g+12."""
    return [g + 4 * h for h in range(NH)]


def _keep(h, i0, j0):
    """Keep score tile (j0, i0) for local head h?  Uses the weakest alibi
    slope among the global heads any core maps to local slot h, so the same
    compiled program is valid on every core."""
    worst_c = min(_c_of(g + 4 * h) for g in range(4))
    dmin = max(0, j0 - (i0 + SI - 1), i0 - (j0 + P - 1))
    return worst_c * dmin <= SKIP_THRESH


def _r(ap):
    return ap.bitcast(F32R)


def _build():
    nc = bacc.Bacc("TRN2", target_bir_lowering=False, debug=False, num_devices=8)

    xb = nc.dram_tensor("xb", [S, D], F32, kind="ExternalInput").ap()
    wq = nc.dram_tensor("wq", [D, INNER], F32, kind="ExternalInput").ap()
    wk = nc.dram_tensor("wk", [D, INNER], F32, kind="ExternalInput").ap()
    wv = nc.dram_tensor("wv", [D, INNER], F32, kind="ExternalInput").ap()
    wo = nc.dram_tensor("wo", [INNER, D], BF16, kind="ExternalInput").ap()
    g8 = nc.dram_tensor("g8", [D], F32, kind="ExternalInput").ap()
    b8 = nc.dram_tensor("b8", [D], F32, kind="ExternalInput").ap()
    kaug_in = nc.dram_tensor("kaug", [2 * NH, S], F16, kind="ExternalInput").ap()
    qaugL_in = nc.dram_tensor("qaugL", [2 * NH, S], F16, kind="ExternalInput").ap()
    qaugU_in = nc.dram_tensor("qaugU", [2 * NH, S], F16, kind="ExternalInput").ap()
    dcor_in = nc.dram_tensor("dcor", [P, 1280], F16, kind="ExternalInput").ap()
    chn_in = nc.dram_tensor("chn", [P, NH], F32, kind="ExternalInput").ap()
    out_d = nc.dram_tensor("out", [S, D], F32, kind="ExternalOutput").ap()

    CH = 1  # LayerNorm chunk: s-tiles resident between stats and apply

    from contextlib import ExitStack
    with tile.TileContext(nc) as tc, ExitStack() as _es:
        consts = _es.enter_context(tc.tile_pool(name="consts", bufs=1))
        wpool = _es.enter_context(tc.tile_pool(name="wpool", bufs=1))
        xnt_pool = _es.enter_context(tc.tile_pool(name="xnt_pool", bufs=1))
        qkpool = _es.enter_context(tc.tile_pool(name="qk", bufs=1))
        vpool = _es.enter_context(tc.tile_pool(name="vpool", bufs=1))
        otpool = _es.enter_context(tc.tile_pool(name="otpool", bufs=1))
        xch = _es.enter_context(tc.tile_pool(name="xch", bufs=4))
        xio = _es.enter_context(tc.tile_pool(name="xio", bufs=2))
        xnp = _es.enter_context(tc.tile_pool(name="xnp", bufs=2))
        small = _es.enter_context(tc.tile_pool(name="small", bufs=4))
        rcp = _es.enter_context(tc.tile_pool(name="rcp", bufs=1))
        ptiles = _es.enter_context(tc.tile_pool(name="ptiles", bufs=6))
        scp = _es.enter_context(tc.tile_pool(name="scp", bufs=2))
        bcp = _es.enter_context(tc.tile_pool(name="bcp", bufs=1))
        psT = _es.enter_context(tc.tile_pool(name="psT", bufs=2, space="PSUM"))
        psP = _es.enter_context(tc.tile_pool(name="psP", bufs=2, space="PSUM"))
        psS = _es.enter_context(tc.tile_pool(name="psS", bufs=3, space="PSUM"))
        psPV = _es.enter_context(tc.tile_pool(name="psPV", bufs=1, space="PSUM"))

        ident = consts.tile([P, P], F32)
        make_identity(nc, ident)
        eps_t = consts.tile([P, 1], F32)
        nc.vector.memset(eps_t, 1e-5)
        gam = consts.tile([P, NDT], F32)
        bet = consts.tile([P, NDT], F32)
        nc.sync.dma_start(out=gam, in_=g8.rearrange("(t p) -> p t", p=P))
        nc.sync.dma_start(out=bet, in_=b8.rearrange("(t p) -> p t", p=P))

        wq_sb = wpool.tile([P, NDT, INNER], F32, tag="wq")
        wk_sb = wpool.tile([P, NDT, INNER], F32, tag="wk")
        wv_sb = wpool.tile([P, NDT, INNER], F32, tag="wv")
        wo_sb = wpool.tile([P, 2, D], BF16, tag="wo")

        dcor = consts.tile([P, 1280], F16)
        chn = consts.tile([P, NH], F32)
        nc.sync.dma_start(out=chn, in_=chn_in)
        nc.sync.dma_start(out=dcor, in_=dcor_in)
        xnt = xnt_pool.tile([P, NDT, S], F32)

        # ------------- LayerNorm (chunked) + transpose to xnT ----------
        # rsqrt(var+eps) is computed on DVE via Newton iteration seeded
        # with reciprocal -- no ScalarE table dependency in this phase.
        for ch in range(NTS // CH):
            xts = []
            mvc = small.tile([P, CH, nc.vector.BN_AGGR_DIM], F32, tag="mvc",
                             name=f"mvc{ch}")
            for t in range(CH):
                st = ch * CH + t
                x_t = xch.tile([P, D], F32, tag="xch", name=f"x{st}")
                nc.sync.dma_start(out=x_t, in_=xb[st * P : st * P + P, :])
                xts.append(x_t)
                stats = small.tile([P, 2, nc.vector.BN_STATS_DIM], F32, tag="stats")
                xr = x_t.rearrange("p (c f) -> p c f", c=2)
                for c in range(2):
                    nc.vector.bn_stats(out=stats[:, c, :], in_=xr[:, c, :])
                nc.vector.bn_aggr(out=mvc[:, t, :], in_=stats)
            v = small.tile([P, CH], F32, tag="veps", name=f"v{ch}")
            nc.vector.tensor_scalar_add(out=v, in0=mvc[:, :, 1], scalar1=1e-5)
            y = small.tile([P, CH], F32, tag="y", name=f"y{ch}")
            nc.vector.reciprocal(out=y, in_=v)
            for it in range(2):
                t1 = small.tile([P, CH], F32, tag="t1", name=f"t1_{ch}_{it}")
                nc.vector.tensor_tensor(out=t1, in0=y, in1=y, op=OP.mult)
                nc.vector.tensor_tensor(out=t1, in0=t1, in1=v, op=OP.mult)
                nc.vector.tensor_scalar(
                    out=t1, in0=t1, scalar1=-0.5, scalar2=1.5,
                    op0=OP.mult, op1=OP.add,
                )
                y2 = small.tile([P, CH], F32, tag="y", name=f"y{ch}_{it}")
                nc.vector.tensor_tensor(out=y2, in0=y, in1=t1, op=OP.mult)
                y = y2
            nb = small.tile([P, CH], F32, tag="nb", name=f"nb{ch}")
            nc.vector.scalar_tensor_tensor(
                out=nb, in0=mvc[:, :, 0], scalar=-1.0, in1=y,
                op0=OP.mult, op1=OP.mult,
            )
            for t in range(CH):
                st = ch * CH + t
                s0 = st * P
                xn_t = xnp.tile([P, D], F32, tag="xn")
                nc.scalar.activation(
                    out=xn_t, in_=xts[t], func=AF.Identity,
                    bias=nb[:, t : t + 1], scale=y[:, t : t + 1],
                )
                for half in range(2):
                    pst = psT.tile([P, SI], F32, tag="psT")
                    for q in range(4):
                        dt = half * 4 + q
                        nc.tensor.transpose(
                            pst[:, q * P : (q + 1) * P],
                            xn_t[:, dt * P : (dt + 1) * P],
                            ident,
                        )
                    for q in range(4):
                        dt = half * 4 + q
                        nc.any.tensor_scalar(
                            out=_r(xnt[:, dt, s0 : s0 + P]),
                            in0=pst[:, q * P : (q + 1) * P],
                            scalar1=gam[:, dt : dt + 1],
                            scalar2=bet[:, dt : dt + 1],
                            op0=OP.mult,
                            op1=OP.add,
                        )

        # ---------------- V projection (+ ones col) -> vaug bf16 -------
        nc.sync.dma_start(out=_r(wv_sb), in_=_r(wv.rearrange("(t p) n -> p t n", p=P)))
        vaug = vpool.tile([P, NTS, NH * (HD + 1)], BF16)
        va4 = vaug.rearrange("p t (h c) -> p t h c", h=NH)
        nc.vector.memset(va4[:, :, :, HD : HD + 1], 1.0)
        for st in range(NTS):
            psv = psP.tile([P, SI], F32, tag="psP")
            for kt in range(NDT):
                nc.tensor.matmul(
                    psv[:, :INNER],
                    _r(xnt[:, kt, st * P : (st + 1) * P]),
                    _r(wv_sb[:, kt, :]),
                    start=(kt == 0),
                    stop=(kt == NDT - 1),
                )
            nc.any.tensor_copy(
                out=va4[:, st, :, 0:HD],
                in_=psv[:, :INNER].rearrange("p (h c) -> p h c", h=NH),
            )

        nc.sync.dma_start(out=_r(wq_sb), in_=_r(wq.rearrange("(t p) n -> p t n", p=P)))
        nc.sync.dma_start(out=_r(wk_sb), in_=_r(wk.rearrange("(t p) n -> p t n", p=P)))
        nc.sync.dma_start(out=wo_sb, in_=wo.rearrange("(t p) n -> p t n", p=P))


        # ------------- Q/K projections -> per-head fp16 tensors --------
        kg = {}
        qL = {}
        qU = {}
        for h in range(NH):
            kg[h] = qkpool.tile([KAUG, S], F16, tag=f"kg{h}", name=f"kg{h}")
            qL[h] = qkpool.tile([KAUG, S], F16, tag=f"qL{h}", name=f"qL{h}")
            qU[h] = qkpool.tile([KAUG, S], F16, tag=f"qU{h}", name=f"qU{h}")
            nc.sync.dma_start(out=kg[h][HD:KAUG, :], in_=kaug_in[2 * h : 2 * h + 2, :])
            nc.sync.dma_start(
                out=qL[h][HD:KAUG, :], in_=qaugL_in[2 * h : 2 * h + 2, :]
            )
            nc.sync.dma_start(
                out=qU[h][HD:KAUG, :], in_=qaugU_in[2 * h : 2 * h + 2, :]
            )
        for pair in range(2):
            hA, hB = 2 * pair, 2 * pair + 1
            for i in range(NI):
                i0 = i * SI
                psk = psP.tile([P, SI], F32, tag="psP")
                for kt in range(NDT):
                    nc.tensor.matmul(
                        psk,
                        _r(wk_sb[:, kt, pair * P : (pair + 1) * P]),
                        _r(xnt[:, kt, i0 : i0 + SI]),
                        start=(kt == 0),
                        stop=(kt == NDT - 1),
                    )
                for h, lo in ((hA, 0), (hB, HD)):
                    nc.any.tensor_copy(
                        out=kg[h][0:HD, i0 : i0 + SI], in_=psk[lo : lo + HD, :]
                    )
        for i in range(NI):
            for pair in range(2):
                hA, hB = 2 * pair, 2 * pair + 1
                i0 = i * SI
                psq = psP.tile([P, SI], F32, tag="psP")
                for kt in range(NDT):
                    nc.tensor.matmul(
                        psq,
                        _r(wq_sb[:, kt, pair * P : (pair + 1) * P]),
                        _r(xnt[:, kt, i0 : i0 + SI]),
                        start=(kt == 0),
                        stop=(kt == NDT - 1),
                    )
                for h, lo in ((hA, 0), (hB, HD)):
                    src_q = psq[lo : lo + HD, :]
                    nc.any.tensor_copy(out=qL[h][0:HD, i0 : i0 + SI], in_=src_q)
                    nc.any.tensor_copy(out=qU[h][0:HD, i0 : i0 + SI], in_=src_q)

        # ------------- attention: scores^T -> exp -> PV ----------------
        outT = otpool.tile([P, 2, S], BF16)
        for i in range(NI):
            i0 = i * SI
            for h in range(NH):
                pts = {}
                for jt in range(NJT):
                    j0 = jt * P
                    if not _keep(h, i0, j0):
                        continue
                    ps = psS.tile([P, SI], F32, tag="psS")
                    if j0 < i0:
                        nc.tensor.matmul(
                            ps, kg[h][:, j0 : j0 + P],
                            qL[h][:, i0 : i0 + SI],
                        )
                    elif j0 >= i0 + SI:
                        nc.tensor.matmul(
                            ps, kg[h][:, j0 : j0 + P],
                            qU[h][:, i0 : i0 + SI],
                        )
                    else:
                        k = (j0 - i0) // P
                        w = (k + 1) * P
                        off = (k * (k + 1) // 2) * P
                        nc.tensor.matmul(
                            ps, kg[h][:, j0 : j0 + P],
                            qL[h][:, i0 : i0 + SI],
                        )
                        sc = scp.tile([P, SI], F32, tag="sc")
                        nc.vector.scalar_tensor_tensor(
                            out=sc[:, :w], in0=dcor[:, off : off + w],
                            scalar=chn[:, h : h + 1], in1=ps[:, :w],
                            op0=OP.mult, op1=OP.add,
                        )
                        if w < SI:
                            nc.any.tensor_copy(out=sc[:, w:], in_=ps[:, w:])
                        pt = ptiles.tile([P, SI], BF16, tag="pt")
                        nc.scalar.activation(
                            out=pt, in_=sc, func=AF.Exp, scale=0.125
                        )
                        pts[jt] = pt
                        continue
                    pt = ptiles.tile([P, SI], BF16, tag="pt")
                    nc.scalar.activation(
                        out=pt, in_=ps, func=AF.Exp, scale=0.125
                    )
                    pts[jt] = pt
                pso = psPV.tile([HD + 1, SI], F32, tag="pv")
                kept = sorted(pts)
                for jt in kept:
                    nc.tensor.matmul(
                        pso,
                        vaug[:, jt, h * (HD + 1) : (h + 1) * (HD + 1)],
                        pts[jt],
                        start=(jt == kept[0]),
                        stop=(jt == kept[-1]),
                    )
                rc = rcp.tile([1, SI], F32, tag="rc")
                nc.vector.reciprocal(out=rc, in_=pso[HD : HD + 1, :])
                bc = bcp.tile([HD, SI], F32, tag="bc")
                nc.gpsimd.partition_broadcast(bc, rc)
                nc.vector.tensor_tensor(
                    out=outT[(h % 2) * HD : (h % 2) * HD + HD, h // 2, i0 : i0 + SI],
                    in0=pso[0:HD, :],
                    in1=bc,
                    op=OP.mult,
                )




        # ---------------- final projection F = out @ Wo ----------------
        for st in range(NTS):
            s0 = st * P
            f_t = xio.tile([P, D], F32, tag="xio")
            for n in range(2):
                psf = psP.tile([P, SI], F32, tag="psP")
                for t in range(2):
                    nc.tensor.matmul(
                        psf,
                        outT[:, t, s0 : s0 + P],
                        wo_sb[:, t, n * SI : (n + 1) * SI],
                        start=(t == 0),
                        stop=(t == 1),
                    )
                nc.vector.tensor_copy(out=f_t[:, n * SI : (n + 1) * SI], in_=psf)
            nc.sync.dma_start(out=out_d[s0 : s0 + P, :], in_=f_t)

    nc.compile()
    return nc


def _core_inputs(x, ln_gamma, ln_beta, Wq, Wk, Wv, Wo):
    """Build the 8 per-core input maps."""
    iota = np.arange(S, dtype=np.float64)
    dcor = np.zeros((P, 1280), dtype=np.float32)
    pp = np.arange(P)
    for k in range(4):
        delta0, w, off = k * P, (k + 1) * P, (k * (k + 1) // 2) * P
        ff = np.arange(w)
        dcor[:, off : off + w] = 16.0 * np.maximum(
            delta0 + pp[:, None] - ff[None, :], 0
        )
    maps = []
    for c in range(8):
        b, hg = c // 4, c % 4
        heads = heads_of_group(hg)
        cols = np.concatenate([np.arange(h * HD, (h + 1) * HD) for h in heads])
        qaugL = np.zeros((2 * NH, S), dtype=np.float64)
        kaug = np.zeros((2 * NH, S), dtype=np.float64)
        chn = np.zeros((P, NH), dtype=np.float32)
        for h in range(NH):
            hgl = heads[h]
            ch = _c_of(hgl)
            kaug[2 * h, :] = iota
            kaug[2 * h + 1, :] = 8.0 * ch
            qaugL[2 * h, :] = 8.0 * ch
            qaugL[2 * h + 1, :] = -iota
            chn[:, h] = -ch
        maps.append(
            {
                "xb": np.ascontiguousarray(x[b]),
                "wq": np.ascontiguousarray(Wq[:, cols]),
                "wk": np.ascontiguousarray(Wk[:, cols]),
                "wv": np.ascontiguousarray(Wv[:, cols]),
                "wo": np.ascontiguousarray(Wo[cols, :]).astype(ml_dtypes.bfloat16),
                "g8": np.ascontiguousarray(ln_gamma),
                "b8": np.ascontiguousarray(ln_beta),
                "kaug": kaug.astype(np.float16),
                "qaugL": qaugL.astype(np.float16),
                "qaugU": (-qaugL).astype(np.float16),
                "dcor": dcor.astype(np.float16),
                "chn": chn,
            }
        )
    return maps


def kernel(x, ln_gamma, ln_beta, Wq, Wk, Wv, Wo, _trace=False):
    x = np.asarray(x, dtype=np.float32)
    if "nc" not in _CACHE:
        _CACHE["nc"] = _build()
    nc = _CACHE["nc"]
    maps = _core_inputs(
        x,
        np.asarray(ln_gamma, np.float32),
        np.asarray(ln_beta, np.float32),
        np.asarray(Wq, np.float32),
        np.asarray(Wk, np.float32),
        np.asarray(Wv, np.float32),
        np.asarray(Wo, np.float32),
    )
    res = run_bass_kernel_spmd(nc, maps, core_ids=list(range(8)), trace=_trace)
    parts = [res.results[c]["out"] for c in range(8)]
    out = np.stack(
        [
            parts[0] + parts[1] + parts[2] + parts[3],
            parts[4] + parts[5] + parts[6] + parts[7],
        ]
    )
    if _trace:
        _CACHE["last_result"] = res
    return out



# revision 17
# speedup vs baseline: 1.4727x; 1.4727x over previous
"""Trainium2 Bass kernel for fused LayerNorm + multi-head ALiBi attention.

Reference computation (B=2, S=2048, D=1024, H=16 heads, dh=64):
    xn = LayerNorm(x) * gamma + beta
    q,k,v = split_heads(xn @ Wq), ... ; att = softmax(q k^T / 8 + alibi); out = (att v) @ Wo

Sharding: 8 cores = 2 batches x 4 head-groups (4 heads each).  Each core
computes a partial output (its heads' contribution through its Wo row-slice);
host sums the 4 partials per batch (the tensor-parallel all-reduce).

v3 design notes:
  - All matmul operands 16-bit (bf16 weights/activations; fp16 score operands
    so the iota alibi-augmentation rows stay exact).  x ships bf16; gamma is
    folded into Wq/Wk/Wv on the host (beta==0 fast path; general affine
    variant compiled lazily if beta != 0).
  - Scores^T tiles [j=128, i<=512], alibi via 2 extra fp16 contraction rows
    (lower/upper variants).  Diagonal-crossing tiles: lower variant plus a
    tensor-engine correction (stationary DCT[k,m]=16*max(m-k,0) x moving
    -c_h*I accumulates -16c*max(p-f,0) over the mixed 128 columns).
  - Per-head alibi band W=30/c restricts score/exp/PV columns per tile
    (psPV memset'd so partial-width accumulation is safe); slot 3 holds all
    four weak heads and runs full-width.
  - Row sums via a ones column in V; normalize = reciprocal_approx_fast +
    gpsimd partition broadcast + DVE multiply straight out of PSUM.
  - Wo for the first half of the sequence is interleaved into the second
    attention block's full-width head so TensorE has independent work while
    ScalarE drains the exp backlog.
"""

import ml_dtypes
import numpy as np

import concourse.bass as bass
import concourse.tile as tile
from concourse import bacc, mybir
from concourse.bass_utils import run_bass_kernel_spmd
from concourse.masks import make_identity

F32 = mybir.dt.float32
BF16 = mybir.dt.bfloat16
F16 = mybir.dt.float16
AF = mybir.ActivationFunctionType
OP = mybir.AluOpType

S = 2048
D = 1024
HD = 64          # head dim
NH = 4           # heads per core
INNER = NH * HD  # 256
P = 128
NTS = S // P     # 16 s-tiles
NDT = D // P     # 8 d-tiles
SI = 512         # i-tile width
NI = S // SI     # 4 i-blocks
NJT = S // P     # 16 j-tiles
KAUG = HD + 2    # 66 = augmented contraction for scores
CHUNKS = (2, 2, 4, 4, 4)  # LayerNorm stats batching (first chunks small)

SKIP_THRESH = 30.0

_CACHE = {}


def _c_of(hgl):
    return 2.0 ** (-8.0 / (16 - hgl))


def heads_of_group(g):
    """Head assignment: core group g takes heads g, g+4, g+8, g+12.  This
    puts all four weak-slope heads {12..15} in local slot 3 (which is full
    -width anyway because of head 15), minimizing total kept-tile area."""
    return [g + 4 * h for h in range(NH)]


def _slot_w():
    ws = []
    for h in range(NH):
        worst_c = min(_c_of(g + 4 * h) for g in range(4))
        ws.append(min(int(SKIP_THRESH / worst_c), S))
    return ws


W_SLOT = _slot_w()


def _rng(h, i0, j0):
    """Column range [a, b) of i-block [i0, i0+SI) touched by j-tile j0 for
    local head slot h.  Tile kept iff a < b."""
    w = W_SLOT[h]
    a = max(i0, j0 - w)
    b = min(i0 + SI, j0 + P + w)
    return a, b


def _build(affine):
    nc = bacc.Bacc("TRN2", target_bir_lowering=False, debug=False, num_devices=8)

    xb = nc.dram_tensor("xb", [S, D], BF16, kind="ExternalInput").ap()
    wq = nc.dram_tensor("wq", [D, INNER], BF16, kind="ExternalInput").ap()
    wk = nc.dram_tensor("wk", [D, INNER], BF16, kind="ExternalInput").ap()
    wv = nc.dram_tensor("wv", [D, INNER], BF16, kind="ExternalInput").ap()
    wo = nc.dram_tensor("wo", [INNER, D], BF16, kind="ExternalInput").ap()
    kaug_in = nc.dram_tensor("kaug", [2 * NH, S], F16, kind="ExternalInput").ap()
    qaugL_in = nc.dram_tensor("qaugL", [2 * NH, S], F16, kind="ExternalInput").ap()
    qaugU_in = nc.dram_tensor("qaugU", [2 * NH, S], F16, kind="ExternalInput").ap()
    cid_in = nc.dram_tensor("cid", [P, NH * P], F16, kind="ExternalInput").ap()
    dct_in = nc.dram_tensor("dct", [P, P], F16, kind="ExternalInput").ap()
    if affine:
        g8 = nc.dram_tensor("g8", [D], F32, kind="ExternalInput").ap()
        b8 = nc.dram_tensor("b8", [D], F32, kind="ExternalInput").ap()
    out_d = nc.dram_tensor("out", [S, D], BF16, kind="ExternalOutput").ap()

    from contextlib import ExitStack
    with tile.TileContext(nc) as tc, ExitStack() as _es:
        consts = _es.enter_context(tc.tile_pool(name="consts", bufs=1))
        wpool = _es.enter_context(tc.tile_pool(name="wpool", bufs=1))
        xnt_pool = _es.enter_context(tc.tile_pool(name="xnt_pool", bufs=1))
        qkpool = _es.enter_context(tc.tile_pool(name="qk", bufs=1))
        vpool = _es.enter_context(tc.tile_pool(name="vpool", bufs=1))
        otpool = _es.enter_context(tc.tile_pool(name="otpool", bufs=1))
        xch = _es.enter_context(tc.tile_pool(name="xch", bufs=6))
        xnp = _es.enter_context(tc.tile_pool(name="xnp", bufs=3))
        small = _es.enter_context(tc.tile_pool(name="small", bufs=4))
        ptiles = _es.enter_context(tc.tile_pool(name="ptiles", bufs=6))
        bcp = _es.enter_context(tc.tile_pool(name="bcp", bufs=4))
        fop = _es.enter_context(tc.tile_pool(name="fop", bufs=3))
        psp = _es.enter_context(tc.tile_pool(name="psp", bufs=5, space="PSUM"))
        psT = _es.enter_context(tc.tile_pool(name="psT", bufs=1, space="PSUM"))
        psPV = _es.enter_context(tc.tile_pool(name="psPV", bufs=2, space="PSUM"))

        ident = consts.tile([P, P], BF16)
        make_identity(nc, ident)
        cid = consts.tile([P, NH * P], F16)
        dct = consts.tile([P, P], F16)
        nc.sync.dma_start(out=cid, in_=cid_in)
        nc.sync.dma_start(out=dct, in_=dct_in)
        if affine:
            gam = consts.tile([P, NDT], F32)
            bet = consts.tile([P, NDT], F32)
            nc.sync.dma_start(out=gam, in_=g8.rearrange("(t p) -> p t", p=P))
            nc.sync.dma_start(out=bet, in_=b8.rearrange("(t p) -> p t", p=P))

        wq_sb = wpool.tile([P, NDT, INNER], BF16, tag="wq")
        wk_sb = wpool.tile([P, NDT, INNER], BF16, tag="wk")
        wv_sb = wpool.tile([P, NDT, INNER], BF16, tag="wv")
        wo_sb = wpool.tile([P, 2, D], BF16, tag="wo")

        xnt = xnt_pool.tile([P, NDT, S], BF16)

        vaug = vpool.tile([P, NTS, NH * (HD + 1)], BF16)
        va4 = vaug.rearrange("p t (h c) -> p t h c", h=NH)
        nc.vector.memset(va4[:, :, :, HD : HD + 1], 1.0)

        kg = {}
        qL = {}
        qU = {}
        for h in range(NH):
            kg[h] = qkpool.tile([KAUG, S], F16, tag=f"kg{h}", name=f"kg{h}")
            qL[h] = qkpool.tile([KAUG, S], F16, tag=f"qL{h}", name=f"qL{h}")
            qU[h] = qkpool.tile([KAUG, S], F16, tag=f"qU{h}", name=f"qU{h}")

        outT = otpool.tile([P, 2, S], BF16)

        # ------------- LayerNorm (chunked) + transpose to xnT + V ------
        st_base = 0
        for ci, chn in enumerate(CHUNKS):
            xts = []
            mvc = small.tile([P, chn, nc.vector.BN_AGGR_DIM], F32, tag=f"mvc{chn}",
                             name=f"mvc{ci}")
            for t in range(chn):
                st = st_base + t
                x_t = xch.tile([P, D], BF16, tag="xch", name=f"x{st}")
                nc.sync.dma_start(out=x_t, in_=xb[st * P : st * P + P, :])
                xts.append(x_t)
                stats = small.tile([P, 2, nc.vector.BN_STATS_DIM], F32, tag="stats")
                xr = x_t.rearrange("p (c f) -> p c f", c=2)
                for c in range(2):
                    nc.vector.bn_stats(out=stats[:, c, :], in_=xr[:, c, :])
                nc.vector.bn_aggr(out=mvc[:, t, :], in_=stats)
            # weight DMAs ride the queue behind the early x tiles
            if ci == 0:
                nc.sync.dma_start(
                    out=wv_sb, in_=wv.rearrange("(t p) n -> p t n", p=P)
                )
            elif ci == 1:
                nc.sync.dma_start(
                    out=wk_sb, in_=wk.rearrange("(t p) n -> p t n", p=P)
                )
                nc.sync.dma_start(
                    out=wq_sb, in_=wq.rearrange("(t p) n -> p t n", p=P)
                )
            elif ci == 2:
                nc.sync.dma_start(out=wo_sb, in_=wo.rearrange("(t p) n -> p t n", p=P))
                for h in range(NH):
                    nc.sync.dma_start(
                        out=kg[h][HD:KAUG, :], in_=kaug_in[2 * h : 2 * h + 2, :]
                    )
                    nc.sync.dma_start(
                        out=qL[h][HD:KAUG, :], in_=qaugL_in[2 * h : 2 * h + 2, :]
                    )
                    nc.sync.dma_start(
                        out=qU[h][HD:KAUG, :], in_=qaugU_in[2 * h : 2 * h + 2, :]
                    )
            # rsqrt(var+eps) via Newton on DVE (seed = reciprocal)
            v = small.tile([P, chn], F32, tag=f"veps{chn}", name=f"v{ci}")
            nc.vector.tensor_scalar_add(out=v, in0=mvc[:, :, 1], scalar1=1e-5)
            y = small.tile([P, chn], F32, tag=f"y{chn}", name=f"y{ci}")
            nc.vector.reciprocal(out=y, in_=v)
            for it in range(2):
                t1 = small.tile([P, chn], F32, tag=f"t1{chn}", name=f"t1_{ci}_{it}")
                nc.vector.tensor_tensor(out=t1, in0=y, in1=y, op=OP.mult)
                nc.vector.tensor_tensor(out=t1, in0=t1, in1=v, op=OP.mult)
                nc.vector.tensor_scalar(
                    out=t1, in0=t1, scalar1=-0.5, scalar2=1.5,
                    op0=OP.mult, op1=OP.add,
                )
                y2 = small.tile([P, chn], F32, tag=f"y{chn}", name=f"y{ci}_{it}")
                nc.vector.tensor_tensor(out=y2, in0=y, in1=t1, op=OP.mult)
                y = y2
            nb = small.tile([P, chn], F32, tag=f"nb{chn}", name=f"nb{ci}")
            nc.vector.scalar_tensor_tensor(
                out=nb, in0=mvc[:, :, 0], scalar=-1.0, in1=y,
                op0=OP.mult, op1=OP.mult,
            )
            for t in range(chn):
                st = st_base + t
                s0 = st * P
                xn_t = xnp.tile([P, D], BF16, tag="xn")
                nc.scalar.activation(
                    out=xn_t, in_=xts[t], func=AF.Identity,
                    bias=nb[:, t : t + 1], scale=y[:, t : t + 1],
                )
                for half in range(2):
                    pst = psT.tile([P, SI], BF16, tag="psT")
                    for q in range(4):
                        dt = half * 4 + q
                        nc.tensor.transpose(
                            pst[:, q * P : (q + 1) * P],
                            xn_t[:, dt * P : (dt + 1) * P],
                            ident,
                        )
                    if affine:
                        for q in range(4):
                            dt = half * 4 + q
                            nc.any.tensor_scalar(
                                out=xnt[:, dt, s0 : s0 + P],
                                in0=pst[:, q * P : (q + 1) * P],
                                scalar1=gam[:, dt : dt + 1],
                                scalar2=bet[:, dt : dt + 1],
                                op0=OP.mult,
                                op1=OP.add,
                            )
                    else:
                        eng = nc.vector if half == 0 else nc.scalar
                        src = pst.rearrange("p (q f) -> p q f", q=4)
                        dst = xnt[:, half * 4 : half * 4 + 4, s0 : s0 + P]
                        if half == 0:
                            nc.vector.tensor_copy(out=dst, in_=src)
                        else:
                            nc.scalar.copy(out=dst, in_=src)
                # V projection for this s-tile
                psv = psp.tile([P, SI], F32, tag="ps", name=f"psv{st}")
                for kt in range(NDT):
                    nc.tensor.matmul(
                        psv[:, :INNER],
                        xnt[:, kt, s0 : s0 + P],
                        wv_sb[:, kt, :],
                        start=(kt == 0),
                        stop=(kt == NDT - 1),
                    )
                nc.scalar.copy(
                    out=va4[:, st, :, 0:HD],
                    in_=psv[:, :INNER].rearrange("p (h c) -> p h c", h=NH),
                )
            st_base += chn

        # ------------- Q/K projections -> per-head fp16 tensors --------
        for i in range(NI):
            i0 = i * SI
            for pair in range(2):
                hA, hB = 2 * pair, 2 * pair + 1
                psk = psp.tile([P, SI], F32, tag="ps", name=f"psk{i}_{pair}")
                for kt in range(NDT):
                    nc.tensor.matmul(
                        psk,
                        wk_sb[:, kt, pair * P : (pair + 1) * P],
                        xnt[:, kt, i0 : i0 + SI],
                        start=(kt == 0),
                        stop=(kt == NDT - 1),
                    )
                for h, lo in ((hA, 0), (hB, HD)):
                    nc.vector.tensor_copy(
                        out=kg[h][0:HD, i0 : i0 + SI], in_=psk[lo : lo + HD, :]
                    )
            for pair in range(2):
                hA, hB = 2 * pair, 2 * pair + 1
                psq = psp.tile([P, SI], F32, tag="ps", name=f"psq{i}_{pair}")
                for kt in range(NDT):
                    nc.tensor.matmul(
                        psq,
                        wq_sb[:, kt, pair * P : (pair + 1) * P],
                        xnt[:, kt, i0 : i0 + SI],
                        start=(kt == 0),
                        stop=(kt == NDT - 1),
                    )
                for h, lo in ((hA, 0), (hB, HD)):
                    nc.scalar.copy(out=qL[h][0:HD, i0 : i0 + SI],
                                   in_=psq[lo : lo + HD, :])
                    nc.vector.tensor_copy(out=qU[h][0:HD, i0 : i0 + SI],
                                          in_=qL[h][0:HD, i0 : i0 + SI])

        # ------------- attention: scores^T -> exp -> PV ----------------
        def emit_head(ip, h, extra=None):
            """Emit scores/exp/PV/normalize for head-slot h over i-blocks
            (2ip, 2ip+1).  `extra` is an optional callable invoked once per
            j-tile step to interleave independent tensor work."""
            pair_is = [2 * ip, 2 * ip + 1]
            kept = {}
            for i in pair_is:
                i0 = i * SI
                kept[i] = [
                    (jt, a, b)
                    for jt in range(NJT)
                    for a, b in [_rng(h, i0, jt * P)]
                    if a < b
                ]
            union_jts = sorted({e[0] for i in pair_is for e in kept[i]})
            full = h == NH - 1
            pso = {}
            for i in pair_is:
                t = psPV.tile([HD + 1, SI], F32, tag="pv",
                              name=f"pv{ip}_{h}_{i}")
                pso[i] = t
                if not full:
                    nc.vector.memset(t, 0.0)
            for jt in union_jts:
                j0 = jt * P
                for i in pair_is:
                    ent = next((e for e in kept[i] if e[0] == jt), None)
                    if ent is None:
                        continue
                    _, a, b = ent
                    i0 = i * SI
                    ps = psp.tile([P, SI], F32, tag="ps",
                                  name=f"ps{ip}_{h}_{i}_{jt}")
                    diag = i0 <= j0 < i0 + SI
                    if diag:
                        if a < j0:
                            nc.tensor.matmul(
                                ps[:, a - i0 : j0 - i0],
                                kg[h][:, j0 : j0 + P],
                                qU[h][:, a:j0],
                                skip_group_check=True,
                            )
                        nc.tensor.matmul(
                            ps[:, j0 - i0 : b - i0],
                            kg[h][:, j0 : j0 + P],
                            qL[h][:, j0:b],
                            start=True,
                            stop=False,
                            skip_group_check=True,
                        )
                        nc.tensor.matmul(
                            ps[:, j0 - i0 : j0 - i0 + P],
                            dct,
                            cid[:, h * P : (h + 1) * P],
                            start=False,
                            stop=True,
                            skip_group_check=True,
                        )
                    elif j0 < i0:
                        nc.tensor.matmul(
                            ps[:, a - i0 : b - i0],
                            kg[h][:, j0 : j0 + P],
                            qL[h][:, a:b],
                        )
                    else:
                        nc.tensor.matmul(
                            ps[:, a - i0 : b - i0],
                            kg[h][:, j0 : j0 + P],
                            qU[h][:, a:b],
                        )
                    pt = ptiles.tile([P, SI], BF16, tag="pt")
                    nc.scalar.activation(
                        out=pt[:, a - i0 : b - i0],
                        in_=ps[:, a - i0 : b - i0],
                        func=AF.Exp, scale=0.125,
                    )
                    last = jt == kept[i][-1][0]
                    vsl = vaug[:, jt, h * (HD + 1) : (h + 1) * (HD + 1)]
                    if full:
                        nc.tensor.matmul(
                            pso[i], vsl, pt,
                            start=(jt == kept[i][0][0]),
                            stop=last,
                        )
                    else:
                        nc.tensor.matmul(
                            pso[i][:, a - i0 : b - i0], vsl,
                            pt[:, a - i0 : b - i0],
                            start=False,
                            stop=last,
                            skip_group_check=True,
                        )
                if extra is not None:
                    extra()
            for idx, i in enumerate(pair_is):
                i0 = i * SI
                zt = bcp.tile([1, SI], F32, tag="zt")
                nc.vector.tensor_copy(out=zt, in_=pso[i][HD : HD + 1, :])
                rc = bcp.tile([1, SI], F32, tag="rc")
                nc.vector.reciprocal_approx_fast(out=rc, in_=zt)
                bc = bcp.tile([HD, SI], F32, tag="bc")
                nc.gpsimd.partition_broadcast(bc, rc)
                nc.vector.tensor_tensor(
                    out=outT[(h % 2) * HD : (h % 2) * HD + HD, h // 2,
                             i0 : i0 + SI],
                    in0=pso[i][0:HD, :],
                    in1=bc,
                    op=OP.mult,
                )

        # Wo unit generator: one (st, n) psum chain per call, DMA after n==1
        def wo_units(st_list):
            for st in st_list:
                s0 = st * P
                f_t = fop.tile([P, D], BF16, tag="fout", name=f"fout{st}")
                for n in range(2):
                    psf = psp.tile([P, SI], F32, tag="ps", name=f"psf{st}_{n}")
                    for t in range(2):
                        nc.tensor.matmul(
                            psf,
                            outT[:, t, s0 : s0 + P],
                            wo_sb[:, t, n * SI : (n + 1) * SI],
                            start=(t == 0),
                            stop=(t == 1),
                        )
                    nc.scalar.copy(out=f_t[:, n * SI : (n + 1) * SI], in_=psf)
                    yield
                nc.sync.dma_start(out=out_d[s0 : s0 + P, :], in_=f_t)

        for h in range(NH):
            emit_head(0, h)
        wo0 = wo_units(range(0, NTS // 2))
        emit_head(1, NH - 1, extra=lambda: next(wo0, None))
        for _ in wo0:
            pass
        for h in range(NH - 1):
            emit_head(1, h)
        for _ in wo_units(range(NTS // 2, NTS)):
            pass

    nc.compile()
    return nc


def _core_inputs(x, ln_gamma, ln_beta, Wq, Wk, Wv, Wo, affine):
    """Build the 8 per-core input maps."""
    iota = np.arange(S, dtype=np.float64)
    dct = 16.0 * np.maximum(
        np.arange(P)[None, :] - np.arange(P)[:, None], 0
    ).astype(np.float32)
    if not affine:
        Wq = ln_gamma[:, None] * Wq
        Wk = ln_gamma[:, None] * Wk
        Wv = ln_gamma[:, None] * Wv
    maps = []
    xb_b = [np.ascontiguousarray(x[b]).astype(ml_dtypes.bfloat16) for b in range(2)]
    for c in range(8):
        b, hg = c // 4, c % 4
        heads = heads_of_group(hg)
        cols = np.concatenate([np.arange(h * HD, (h + 1) * HD) for h in heads])
        qaugL = np.zeros((2 * NH, S), dtype=np.float64)
        kaug = np.zeros((2 * NH, S), dtype=np.float64)
        cid = np.zeros((P, NH * P), dtype=np.float32)
        for h in range(NH):
            hgl = heads[h]
            chd = _c_of(hgl)
            kaug[2 * h, :] = iota
            kaug[2 * h + 1, :] = 8.0 * chd
            qaugL[2 * h, :] = 8.0 * chd
            qaugL[2 * h + 1, :] = -iota
            cid[:, h * P : (h + 1) * P] = -chd * np.eye(P, dtype=np.float32)
        m = {
            "xb": xb_b[b],
            "wq": np.ascontiguousarray(Wq[:, cols]).astype(ml_dtypes.bfloat16),
            "wk": np.ascontiguousarray(Wk[:, cols]).astype(ml_dtypes.bfloat16),
            "wv": np.ascontiguousarray(Wv[:, cols]).astype(ml_dtypes.bfloat16),
            "wo": np.ascontiguousarray(Wo[cols, :]).astype(ml_dtypes.bfloat16),
            "kaug": kaug.astype(np.float16),
            "qaugL": qaugL.astype(np.float16),
            "qaugU": (-qaugL).astype(np.float16),
            "cid": cid.astype(np.float16),
            "dct": dct.astype(np.float16),
        }
        if affine:
            m["g8"] = np.ascontiguousarray(ln_gamma)
            m["b8"] = np.ascontiguousarray(ln_beta)
        maps.append(m)
    return maps


def kernel(x, ln_gamma, ln_beta, Wq, Wk, Wv, Wo, _trace=False):
    x = np.asarray(x, dtype=np.float32)
    ln_gamma = np.asarray(ln_gamma, np.float32)
    ln_beta = np.asarray(ln_beta, np.float32)
    affine = bool(np.any(ln_beta))
    key = ("nc", affine)
    if key not in _CACHE:
        _CACHE[key] = _build(affine)
    nc = _CACHE[key]
    maps = _core_inputs(
        x,
        ln_gamma,
        ln_beta,
        np.asarray(Wq, np.float32),
        np.asarray(Wk, np.float32),
        np.asarray(Wv, np.float32),
        np.asarray(Wo, np.float32),
        affine,
    )
    res = run_bass_kernel_spmd(nc, maps, core_ids=list(range(8)), trace=_trace)
    parts = [np.asarray(res.results[c]["out"], dtype=np.float32) for c in range(8)]
    out = np.stack(
        [
            parts[0] + parts[1] + parts[2] + parts[3],
            parts[4] + parts[5] + parts[6] + parts[7],
        ]
    )
    if _trace:
        _CACHE["last_result"] = res
    return out
